# revision 1
# baseline (speedup 1.0000x reference)
"""Trainium2 Bass kernel for nn_MeanStdStiffRegularizer (segment reduce).

Strategy (8 NeuronCores, data-parallel over edges):
  - Each core gets 1/8 of the edges laid out as [128, 16384].
  - Per 128-edge column f, a PE matmul scatters values into PSUM bins:
      lhsT = one-hot of (idx & 63)   -> 64 PSUM partitions (bins)
      rhs  = 8 hi-group masks (idx >> 6) x 4 value streams -> 32 psum cols
    PSUM [64, 32] accumulates every per-segment sum for 512 segments.
  - The PE array runs in 128x32 column-tiling mode: 4 independent tiles,
    each accumulating every 4th edge column into its own PSUM bank.
  - The bin one-hot is built TRANSPOSED ([P, 64, F]) with 64 tensor_scalar
    is_equal ops (contiguous step-1 16-bit in/out -> DVE 4x mode); the
    matmul streams it as its (strided-column) moving operand.
  - Value streams: bf16 x, log(|x|+eps), log^2, count (exact); the rhs
    fold (hi-mask x value kron) uses pair-duplicated masks so the DVE
    reads step-1 16-bit pairs.
  - The [512 segments x 4 sums] partials are summed across cores and the
    final mean/std losses are computed on host in float64.
"""

import sys
import types

import numpy as np

N_EDGES = 16777216
NUM_SEG = 512
STRENGTH = 0.01
STD_WEIGHT = 0.5
EPS = 1e-6

N_CORES = 8
P = 128
F_TOT = N_EDGES // N_CORES // P  # 16384 edges per partition per core
F_MACRO = 512
N_BIN = 64   # idx & 63 -> psum partitions
N_HI = 8     # idx >> 6 -> rhs groups
N_ST = 4     # value streams: x, log, log^2, count
N_COL = N_HI * N_ST  # 32 psum columns
N_PETILE = 4  # PE array column tiles (128x32 mode)
GPS_BINS = 0  # one-hot bins on GpSimd: its SBUF-port sharing starves DVE
ACT_BINS = 0  # one-hot bins on Scalar engine: 4 measured slower (ACT-bound)


def _install_ntff_hook():
    """Register the axon NTFF profiling hook (missing antenv.axon_hooks)."""
    if "antenv.axon_hooks" in sys.modules:
        return
    mod = types.ModuleType("antenv.axon_hooks")
    _h = [None]
    mod.set_axon_ntff_profile_hook = lambda h: _h.__setitem__(0, h)
    mod.get_axon_ntff_profile_hook = lambda: _h[0]
    sys.modules["antenv.axon_hooks"] = mod
    try:
        from trn_agent_boot.trn_boot import _ntff_profile_via_ctypes

        mod.set_axon_ntff_profile_hook(
            _ntff_profile_via_ctypes("/opt/axon/libaxon_pjrt.so")
        )
    except Exception:
        pass


_NO_SPLIT_OPCODES = {
    "CollectiveCompute",
}


def _split_sync_waits(bir_json_bytes):
    """Rewrite BIR so no TPB instruction carries more than one sync wait.

    The walrus codegen in this container supports a single sync-wait slot
    per TPB instruction ("Too many sync wait commands" otherwise).  Extra
    waits are hoisted onto EventSemaphore instructions inserted immediately
    before, on the same engine (same issue-gating semantics).
    """
    import json

    j = json.loads(bir_json_bytes)
    n_split = 0
    uid = [0]
    for f in j["functions"]:
        for b in f["blocks"]:
            out = []
            for ins in b["instructions"]:
                si = ins.get("sync_info")
                ow = (si or {}).get("on_wait") or []
                if len(ow) > 1 and ins.get("opcode") not in _NO_SPLIT_OPCODES:
                    for w in ow[:-1]:
                        uid[0] += 1
                        out.append(
                            {
                                "debug": ins.get("debug", 0),
                                "engine": ins["engine"],
                                "ins": [],
                                "name": f"{ins['name']}-wsplit{uid[0]}",
                                "opcode": "EventSemaphore",
                                "outs": [],
                                "sync_info": {"on_update": [], "on_wait": [w]},
                            }
                        )
                    si["on_wait"] = [ow[-1]]
                    n_split += 1
                out.append(ins)
            b["instructions"] = out
    return json.dumps(j).encode(), n_split


def build_nc(f_tot=F_TOT, f_macro=F_MACRO, n_cores=N_CORES):
    """Build the per-core Bass program (SPMD: same program on every core)."""
    import concourse.bass as bass
    import concourse.tile as tile
    from concourse import mybir

    f32 = mybir.dt.float32
    bf16 = mybir.dt.bfloat16
    i16 = mybir.dt.int16
    AOP = mybir.AluOpType
    ACT = mybir.ActivationFunctionType

    assert f_tot % f_macro == 0

    nc = bass.Bass(
        "TRN2", target_bir_lowering=False, debug=False, num_devices=n_cores
    )
    x_d = nc.dram_tensor("x", [P, f_tot], f32, kind="ExternalInput")
    lo6_d = nc.dram_tensor("lo6", [P, f_tot], i16, kind="ExternalInput")
    hi3_d = nc.dram_tensor("hi3", [P, f_tot], i16, kind="ExternalInput")
    out_d = nc.dram_tensor(
        "out", [N_PETILE * N_COL, N_BIN], f32, kind="ExternalOutput"
    )

    n_macro = f_tot // f_macro

    with tile.TileContext(nc) as tc:
        with (
            tc.tile_pool(name="const", bufs=1) as cpool,
            tc.tile_pool(name="io", bufs=2) as io,
            tc.tile_pool(name="mid", bufs=2) as mid,
            tc.tile_pool(name="oh", bufs=2) as ohp,
            tc.tile_pool(name="rh", bufs=2) as rhp,
            tc.tile_pool(name="fin", bufs=1) as fin,
            tc.tile_pool(name="acc", bufs=1, space="PSUM") as psum,
        ):
            eps_t = cpool.tile([P, 1], f32)
            nc.vector.memset(eps_t[:], EPS)
            negg = cpool.tile([P, N_HI], f32)
            for g in range(N_HI):
                nc.vector.memset(negg[:, g : g + 1], float(-g))
            if ACT_BINS:
                negb = cpool.tile([P, ACT_BINS], f32)
                for i in range(ACT_BINS):
                    nc.vector.memset(
                        negb[:, i : i + 1], float(-(N_BIN - ACT_BINS + i))
                    )

            # 4 independent 128x32 PE column tiles, each accumulating every
            # 4th f-column into its own PSUM bank (own 32-partition window).
            accs = []
            for q in range(N_PETILE):
                acc_q = psum.tile([P, N_BIN], f32, tag=f"acc{q}", name=f"acc{q}")
                accs.append(acc_q)

            mm_q = [0] * N_PETILE
            total_q = f_tot // N_PETILE
            for t in range(n_macro):
                ts = slice(t * f_macro, (t + 1) * f_macro)
                xt = io.tile([P, f_macro], f32, tag="xt")
                nc.sync.dma_start(xt[:], x_d[:, ts])
                lo6 = io.tile([P, f_macro], i16, tag="lo6")
                nc.sync.dma_start(lo6[:], lo6_d[:, ts])
                hi3 = io.tile([P, f_macro], i16, tag="hi3")
                nc.sync.dma_start(hi3[:], hi3_d[:, ts])

                ax = mid.tile([P, f_macro], f32, tag="ax")
                nc.scalar.activation(ax[:], xt[:], ACT.Abs)
                lx = mid.tile([P, f_macro], f32, tag="lx")
                nc.scalar.activation(lx[:], ax[:], ACT.Ln, bias=eps_t[:])
                qx = ax  # reuse: ax is dead after Ln
                nc.scalar.activation(qx[:], lx[:], ACT.Square)

                # value streams, f-major: vv[:, f, j] (strided ACT writes)
                vv = mid.tile([P, f_macro, N_ST], bf16, tag="vv")
                nc.scalar.activation(vv[:, :, 0], xt[:], ACT.Copy)
                nc.scalar.activation(vv[:, :, 1], lx[:], ACT.Copy)
                nc.scalar.activation(vv[:, :, 2], qx[:], ACT.Copy)
                nc.vector.memset(vv[:, :, 3], 1.0)

                # hi-group masks, f-major and pair-duplicated along a trailing
                # size-2 axis so the rhs fold reads step-1 pairs (2x mode):
                # m8d[:, f, g, u] = (hi3[f] == g) for u in {0, 1}.
                # Built on the otherwise-idle Scalar engine with the exact
                # integer identity  1[u == g] = relu(1 - (u - g)^2).
                m8d = mid.tile([P, f_macro, N_HI, 2], bf16, tag="m8d")
                for g in range(N_HI):
                    tg = mid.tile([P, f_macro], f32, tag="tg")
                    nc.scalar.activation(
                        tg[:], hi3[:], ACT.Square, bias=negg[:, g : g + 1]
                    )
                    nc.scalar.activation(
                        m8d[:, :, g, :],
                        tg[:].unsqueeze(2).broadcast_to([P, f_macro, 2]),
                        ACT.Relu,
                        bias=1.0,
                        scale=-1.0,
                    )

                # transposed one-hot of (idx & 63): ohT[:, b, :] contiguous
                # (two-scalar tensor_scalar fuses the mask: 4x on DVE)
                ohT = ohp.tile([P, N_BIN, f_macro], bf16, tag="ohT")
                for b in range(N_BIN - ACT_BINS):
                    nc.vector.tensor_scalar(
                        ohT[:, b, :], lo6[:], b, None, AOP.is_equal
                    )
                for i in range(ACT_BINS):
                    b = N_BIN - ACT_BINS + i
                    tb = mid.tile([P, f_macro], f32, tag="tg")
                    nc.scalar.activation(
                        tb[:], lo6[:], ACT.Square, bias=negb[:, i : i + 1]
                    )
                    nc.scalar.activation(
                        ohT[:, b, :], tb[:], ACT.Relu, bias=1.0, scale=-1.0
                    )

                # rhs values, f-major: rh[:, f, g, j] = m8[:, f, g]*vv[:, f, j]
                # so the matmul's stationary operand rh[:, fi, :, :] is
                # contiguous.  Small-stride broadcast APs keep this at ~1x.
                f_chunk = f_macro // 4 if f_macro >= 512 else f_macro
                for c0 in range(0, f_macro, f_chunk):
                    cs = slice(c0, c0 + f_chunk)
                    rh = rhp.tile([P, f_chunk, N_HI, N_ST], bf16, tag="rh")
                    nc.vector.tensor_tensor(
                        rh[:].rearrange(
                            "p f g (a u) -> p f g a u", a=N_ST // 2
                        ),
                        m8d[:, cs, :, :]
                        .unsqueeze(3)
                        .broadcast_to([P, f_chunk, N_HI, N_ST // 2, 2]),
                        vv[:, cs, :]
                        .rearrange("p f (a u) -> p f a u", a=N_ST // 2)
                        .unsqueeze(2)
                        .broadcast_to([P, f_chunk, N_HI, N_ST // 2, 2]),
                        AOP.mult,
                    )

                    for fi in range(f_chunk):
                        q = fi % N_PETILE
                        nc.tensor.matmul(
                            accs[q][q * N_COL : (q + 1) * N_COL, :],
                            rh[:, fi, :, :],
                            ohT[:, :, c0 + fi],
                            start=(mm_q[q] == 0),
                            stop=(mm_q[q] == total_q - 1),
                            tile_position=(0, q * N_COL),
                        )
                        mm_q[q] += 1

            outsb = fin.tile([P, N_BIN], f32)
            for q in range(N_PETILE):
                sl = slice(q * N_COL, (q + 1) * N_COL)
                nc.vector.tensor_copy(outsb[sl, :], accs[q][sl, :])
            nc.sync.dma_start(out_d[:], outsb[:])

    return nc


_PROG_CACHE = {}


def _get_prog(f_tot=F_TOT, f_macro=F_MACRO):
    key = (f_tot, f_macro)
    if key not in _PROG_CACHE:
        nc = build_nc(f_tot, f_macro)
        fixed, _n = _split_sync_waits(nc.to_json_bytes())
        nc.to_json_bytes = lambda: fixed
        _PROG_CACHE[key] = nc
    return _PROG_CACHE[key]


def _finale(partials, target_mean, target_std):
    """partials: [512, 4] float64 summed across cores -> scalar loss."""
    xs = partials[:, 0]
    ls = partials[:, 1]
    qs = partials[:, 2]
    cnt = partials[:, 3]
    cg = np.maximum(cnt, 1.0)
    mean_w = xs / cg
    mean_log = ls / cg
    log_var = qs / cg - mean_log**2
    std_w = np.sqrt(log_var + EPS)
    mean_loss = np.mean((mean_w - target_mean.astype(np.float64)) ** 2)
    std_loss = np.mean((std_w - target_std.astype(np.float64)) ** 2)
    total = (1.0 - STD_WEIGHT) * mean_loss + STD_WEIGHT * std_loss
    return np.float32(total * STRENGTH)


def run_partials(x, idx, trace=False):
    """Run the device program; return [512, 4] partials summed over cores."""
    _install_ntff_hook()
    from concourse.bass_utils import run_bass_kernel_spmd

    nc = _get_prog()
    x = np.asarray(x, dtype=np.float32)
    idx = np.asarray(idx)
    per_core = N_EDGES // N_CORES
    in_maps = []
    for c in range(N_CORES):
        sl = slice(c * per_core, (c + 1) * per_core)
        idx_c = idx[sl].reshape(P, F_TOT).astype(np.int16)
        in_maps.append(
            {
                "x": np.ascontiguousarray(x[sl].reshape(P, F_TOT)),
                "lo6": np.ascontiguousarray(idx_c & np.int16(63)),
                "hi3": np.ascontiguousarray(idx_c >> np.int16(6)),
            }
        )
    res = run_bass_kernel_spmd(
        nc, in_maps, list(range(N_CORES)), trace=trace
    )
    # out[q*32 + g*N_ST + j, b] holds the PE-tile-q partial sums for
    # segment s = g*64 + b, stream j; sum over q and cores.
    partials = np.zeros((NUM_SEG, N_ST), dtype=np.float64)
    for c in range(N_CORES):
        o = res.results[c]["out"].astype(np.float64)  # [128, 64]
        o = o.reshape(N_PETILE, N_HI, N_ST, N_BIN).sum(axis=0)
        partials += o.transpose(0, 2, 1).reshape(NUM_SEG, N_ST)
    return partials, res


def kernel(x, idx, target_mean, target_std):
    partials, _res = run_partials(x, idx, trace=False)
    return _finale(
        partials, np.asarray(target_mean), np.asarray(target_std)
    )



# revision 4
# speedup vs baseline: 25.8700x; 25.8700x over previous
"""Trainium2 Bass kernel for nn_MeanStdStiffRegularizer (segment reduce).

Strategy (8 NeuronCores, segment-sharded, sort-based):
  - The host shards BY SEGMENT: core c owns segments [64c, 64c+64).  Edges
    are permuted (stable sort by (segment, sign)) and padded so that each
    (segment, sign) group occupies a fixed [128, 136] block of the per-core
    [128, 17408] bf16 image; column f = t*128 + (2*seg_local + sign).
    Only |x| is shipped (sign is encoded in the column parity), so no idx
    tensor and no abs op on device.  Pads are 1.0 (ln(1+eps) ~ 0) and are
    subtracted exactly on the host.
  - Device per core: Ln(|x|+eps) on ScalarE (one 1x pass), L^2 on VectorE
    (bf16 tensor_tensor 2x), and all segment sums on the PE: matmul with a
    ones[128,1] stationary against 512-wide moving slabs, accumulated in
    PSUM.  Four PE column strips (tile_position) run concurrently; each
    PSUM column j accumulates group g = j%128.
  - No collective: each core returns 3x[128,512] f32 partials; the host
    folds strips/replicas and does the final 512-sized math in float64.
"""

import sys
import types

import numpy as np

N_EDGES = 16777216
NUM_SEG = 512
STRENGTH = 0.01
STD_WEIGHT = 0.5
EPS = 1e-6

N_CORES = 8
P = 128
SEG_PER_CORE = NUM_SEG // N_CORES  # 64
N_GRP = 2 * SEG_PER_CORE  # 128 (seg, sign) groups per core
TPP = 136  # elems per partition per (seg, sign) group
C2 = P * TPP  # 17408 capacity per (seg, sign) group
F_TOT = N_GRP * TPP  # 17408 free elems per partition
SLAB = 512
N_STRIP = 4
TILES = (4096, 4096, 4096, 4096, 1024)
assert sum(TILES) == F_TOT


def _install_ntff_hook():
    """Register the axon NTFF profiling hook (missing antenv.axon_hooks)."""
    if "antenv.axon_hooks" in sys.modules:
        return
    mod = types.ModuleType("antenv.axon_hooks")
    _h = [None]
    mod.set_axon_ntff_profile_hook = lambda h: _h.__setitem__(0, h)
    mod.get_axon_ntff_profile_hook = lambda: _h[0]
    sys.modules["antenv.axon_hooks"] = mod
    try:
        from trn_agent_boot.trn_boot import _ntff_profile_via_ctypes

        mod.set_axon_ntff_profile_hook(
            _ntff_profile_via_ctypes("/opt/axon/libaxon_pjrt.so")
        )
    except Exception:
        pass


_NO_SPLIT_OPCODES = {
    "CollectiveCompute",
}


def _split_sync_waits(bir_json_bytes):
    """Rewrite BIR so no TPB instruction carries more than one sync wait.

    The walrus codegen in this container supports a single sync-wait slot
    per TPB instruction ("Too many sync wait commands" otherwise).  Extra
    waits are hoisted onto EventSemaphore instructions inserted immediately
    before, on the same engine (same issue-gating semantics).
    """
    import json

    j = json.loads(bir_json_bytes)
    n_split = 0
    uid = [0]
    for f in j["functions"]:
        for b in f["blocks"]:
            out = []
            for ins in b["instructions"]:
                si = ins.get("sync_info")
                ow = (si or {}).get("on_wait") or []
                if len(ow) > 1 and ins.get("opcode") not in _NO_SPLIT_OPCODES:
                    for w in ow[:-1]:
                        uid[0] += 1
                        out.append(
                            {
                                "debug": ins.get("debug", 0),
                                "engine": ins["engine"],
                                "ins": [],
                                "name": f"{ins['name']}-wsplit{uid[0]}",
                                "opcode": "EventSemaphore",
                                "outs": [],
                                "sync_info": {"on_update": [], "on_wait": [w]},
                            }
                        )
                    si["on_wait"] = [ow[-1]]
                    n_split += 1
                out.append(ins)
            b["instructions"] = out
    return json.dumps(j).encode(), n_split


def build_nc(n_cores=N_CORES):
    """Build the per-core Bass program (SPMD: same program on every core)."""
    import concourse.bass as bass
    import concourse.tile as tile
    from concourse import mybir

    f32 = mybir.dt.float32
    bf16 = mybir.dt.bfloat16
    AOP = mybir.AluOpType
    ACT = mybir.ActivationFunctionType

    nc = bass.Bass(
        "TRN2", target_bir_lowering=False, debug=False, num_devices=n_cores
    )
    xs_d = nc.dram_tensor("xs", [P, F_TOT], bf16, kind="ExternalInput")
    out_d = nc.dram_tensor("out", [P, 3 * SLAB], f32, kind="ExternalOutput")

    n_slab = F_TOT // SLAB  # 34
    # per (stream, strip) matmul totals for start/stop flags
    strip_total = [0] * N_STRIP
    for i in range(n_slab):
        strip_total[i % N_STRIP] += 1

    with tile.TileContext(nc) as tc:
        with (
            tc.tile_pool(name="const", bufs=1) as cpool,
            tc.tile_pool(name="io", bufs=2) as io,
            tc.tile_pool(name="mid", bufs=2) as mid,
            tc.tile_pool(name="fin", bufs=1) as fin,
            tc.tile_pool(name="acc", bufs=1, space="PSUM") as psum,
        ):
            ones = cpool.tile([P, 1], bf16)
            nc.vector.memset(ones[:], 1.0)
            eps_t = cpool.tile([P, 1], f32)
            nc.vector.memset(eps_t[:], EPS)

            accs = [
                psum.tile([P, SLAB], f32, tag=f"acc{s}", name=f"acc{s}")
                for s in range(3)
            ]

            nmm = [[0] * N_STRIP for _ in range(3)]
            slab_idx = 0
            f0 = 0
            for ti, fm in enumerate(TILES):
                xt = io.tile([P, fm], bf16, tag=f"xt{fm}")
                nc.sync.dma_start(xt[:], xs_d[:, f0 : f0 + fm])
                lx = mid.tile([P, fm], bf16, tag=f"lx{fm}")
                nc.scalar.activation(lx[:], xt[:], ACT.Ln, bias=eps_t[:])
                sq = mid.tile([P, fm], bf16, tag=f"sq{fm}")
                nc.vector.tensor_tensor(sq[:], lx[:], lx[:], AOP.mult)

                for j in range(fm // SLAB):
                    sl = slice(j * SLAB, (j + 1) * SLAB)
                    q = slab_idx % N_STRIP
                    for s, src in enumerate((xt, lx, sq)):
                        nc.tensor.matmul(
                            accs[s][32 * q : 32 * q + 1, :],
                            ones[:],
                            src[:, sl],
                            start=(nmm[s][q] == 0),
                            stop=(nmm[s][q] == strip_total[q] - 1),
                            tile_position=(0, 32 * q),
                        )
                        nmm[s][q] += 1
                    slab_idx += 1
                f0 += fm

            outsb = fin.tile([P, 3 * SLAB], f32)
            nc.vector.tensor_copy(outsb[:, 0:SLAB], accs[0][:, :])
            nc.scalar.activation(
                outsb[:, SLAB : 2 * SLAB], accs[1][:, :], ACT.Copy
            )
            nc.vector.tensor_copy(outsb[:, 2 * SLAB : 3 * SLAB], accs[2][:, :])
            nc.sync.dma_start(out_d[:], outsb[:])

    return nc


_PROG_CACHE = {}


def _get_prog():
    key = 0
    if key not in _PROG_CACHE:
        nc = build_nc()
        fixed, _n = _split_sync_waits(nc.to_json_bytes())
        nc.to_json_bytes = lambda: fixed
        _PROG_CACHE[key] = nc
    return _PROG_CACHE[key]


def _prepare(x, idx):
    """Sort/pad edges into per-core [128, F_TOT] |x| bf16 images.

    Returns (in_maps, host state dict for the finale).
    """
    import ml_dtypes

    x = np.asarray(x, dtype=np.float32).ravel()
    idx = np.asarray(idx).ravel().astype(np.int64)
    n = x.shape[0]

    neg = (x < 0).astype(np.int64)
    key = idx * 2 + neg
    order = np.argsort(key, kind="stable")
    xs = x[order]
    ks = key[order]
    gcnt = np.bincount(key, minlength=2 * NUM_SEG)
    gstart = np.zeros(2 * NUM_SEG, dtype=np.int64)
    np.cumsum(gcnt[:-1], out=gstart[1:])
    rank = np.arange(n, dtype=np.int64) - gstart[ks]
    ok = rank < C2

    flat = np.ones(2 * NUM_SEG * C2, dtype=np.float32)
    flat[ks[ok] * C2 + rank[ok]] = np.abs(xs[ok])

    # exact host-side corrections (float64)
    spill_x = np.zeros(NUM_SEG, dtype=np.float64)
    spill_l = np.zeros(NUM_SEG, dtype=np.float64)
    spill_q = np.zeros(NUM_SEG, dtype=np.float64)
    if not ok.all():
        sp = ~ok
        seg_sp = (ks[sp] >> 1).astype(np.int64)
        xv = xs[sp].astype(np.float64)
        lv = np.log(np.abs(xv) + EPS)
        np.add.at(spill_x, seg_sp, xv)
        np.add.at(spill_l, seg_sp, lv)
        np.add.at(spill_q, seg_sp, lv * lv)

    npad = (C2 - np.minimum(gcnt, C2)).astype(np.float64)  # [1024]
    counts = np.bincount(idx, minlength=NUM_SEG).astype(np.float64)

    flat16 = flat.astype(ml_dtypes.bfloat16)
    padded = flat16.reshape(NUM_SEG, 2, P, TPP)
    in_maps = []
    for c in range(N_CORES):
        a = padded[c * SEG_PER_CORE : (c + 1) * SEG_PER_CORE]  # [64,2,128,136]
        img = np.ascontiguousarray(
            a.transpose(2, 3, 0, 1).reshape(P, F_TOT)
        )
        in_maps.append({"xs": img})

    state = {
        "npad": npad.reshape(NUM_SEG, 2),
        "counts": counts,
        "spill": (spill_x, spill_l, spill_q),
    }
    return in_maps, state


def _fold_outputs(results):
    """Per-core [128, 1536] f32 -> [3, NUM_SEG, 2] (stream, seg, sign)."""
    sums = np.zeros((3, NUM_SEG, 2), dtype=np.float64)
    rows = (0, 32, 64, 96)
    for c, res in enumerate(results):
        o = np.asarray(res["out"], dtype=np.float64)  # [128, 1536]
        for s in range(3):
            m = o[:, s * SLAB : (s + 1) * SLAB]
            v = m[list(rows)].sum(axis=0)  # [512] psum columns
            g = v.reshape(SLAB // N_GRP, N_GRP).sum(axis=0)  # [128] groups
            seg0 = c * SEG_PER_CORE
            sums[s, seg0 : seg0 + SEG_PER_CORE, :] = g.reshape(
                SEG_PER_CORE, 2
            )
    return sums


def _finale(sums, state, target_mean, target_std):
    npad = state["npad"]  # [512, 2]
    counts = state["counts"]
    spill_x, spill_l, spill_q = state["spill"]

    lpad = np.log1p(EPS)
    sx = (
        (sums[0, :, 0] - npad[:, 0])
        - (sums[0, :, 1] - npad[:, 1])
        + spill_x
    )
    sl = sums[1].sum(axis=1) - npad.sum(axis=1) * lpad + spill_l
    sq = sums[2].sum(axis=1) - npad.sum(axis=1) * lpad * lpad + spill_q

    cg = np.maximum(counts, 1.0)
    mean_w = sx / cg
    mean_log = sl / cg
    log_var = sq / cg - mean_log**2
    std_w = np.sqrt(log_var + EPS)
    tm = np.asarray(target_mean, dtype=np.float64)
    ts = np.asarray(target_std, dtype=np.float64)
    mean_loss = np.mean((mean_w - tm) ** 2)
    std_loss = np.mean((std_w - ts) ** 2)
    total = (1.0 - STD_WEIGHT) * mean_loss + STD_WEIGHT * std_loss
    return np.float32(total * STRENGTH)


def run_device(x, idx, trace=False):
    """Run the device program; returns (sums, state, res)."""
    _install_ntff_hook()
    from concourse.bass_utils import run_bass_kernel_spmd

    nc = _get_prog()
    in_maps, state = _prepare(x, idx)
    res = run_bass_kernel_spmd(
        nc, in_maps, list(range(N_CORES)), trace=trace
    )
    sums = _fold_outputs(res.results)
    return sums, state, res


def kernel(x, idx, target_mean, target_std):
    sums, state, _res = run_device(x, idx, trace=False)
    return _finale(sums, state, target_mean, target_std)


# revision 9
# speedup vs baseline: 26.9619x; 1.0422x over previous
"""Trainium2 Bass kernel for nn_MeanStdStiffRegularizer (segment reduce).

Strategy (8 NeuronCores, segment-sharded, sort-based):
  - The host shards BY SEGMENT: core c owns segments [64c, 64c+64).  Edges
    are permuted (stable sort by (segment, sign)) and padded so that each
    (segment, sign) group occupies a fixed [128, 136] block of the per-core
    [128, 17408] bf16 image; column f = t*128 + (2*seg_local + sign).
    Only |x| is shipped (sign is encoded in the column parity), so no idx
    tensor and no abs op on device.  Pads are 1.0 (ln(1+eps) ~ 0) and are
    subtracted exactly on the host.
  - Device per core: Ln(|x|+eps) on ScalarE (one 1x pass), L^2 on VectorE
    (bf16 tensor_tensor 2x), and all segment sums on the PE: matmul with a
    ones[128,1] stationary against 512-wide moving slabs, accumulated in
    PSUM.  Four PE column strips (tile_position) run concurrently; each
    PSUM column j accumulates group g = j%128.
  - No collective: each core returns 3x[128,512] f32 partials; the host
    folds strips/replicas and does the final 512-sized math in float64.
"""

import sys
import types

import numpy as np

N_EDGES = 16777216
NUM_SEG = 512
STRENGTH = 0.01
STD_WEIGHT = 0.5
EPS = 1e-6

N_CORES = 8
P = 128
SEG_PER_CORE = NUM_SEG // N_CORES  # 64
N_GRP = 2 * SEG_PER_CORE  # 128 (seg, sign) groups per core
TPP = 132  # elems per partition per (seg, sign) group
C2 = P * TPP  # 16896 capacity per (seg, sign) group
F_TOT = N_GRP * TPP  # 16896 free elems per partition
SLAB = 512
N_STRIP = 4
# DMA/ACT chunk boundaries (slab-aligned; small first chunk for fast ramp,
# small last chunk for a short tail)
CHUNKS = (512, 1024, 2048, 4096, 4096, 4096, 1024)
assert sum(CHUNKS) == F_TOT


def _install_ntff_hook():
    """Register the axon NTFF profiling hook (missing antenv.axon_hooks)."""
    if "antenv.axon_hooks" in sys.modules:
        return
    mod = types.ModuleType("antenv.axon_hooks")
    _h = [None]
    mod.set_axon_ntff_profile_hook = lambda h: _h.__setitem__(0, h)
    mod.get_axon_ntff_profile_hook = lambda: _h[0]
    sys.modules["antenv.axon_hooks"] = mod
    try:
        from trn_agent_boot.trn_boot import _ntff_profile_via_ctypes

        mod.set_axon_ntff_profile_hook(
            _ntff_profile_via_ctypes("/opt/axon/libaxon_pjrt.so")
        )
    except Exception:
        pass


_NO_SPLIT_OPCODES = {
    "CollectiveCompute",
}


def _split_sync_waits(bir_json_bytes):
    """Rewrite BIR so no TPB instruction carries more than one sync wait.

    The walrus codegen in this container supports a single sync-wait slot
    per TPB instruction ("Too many sync wait commands" otherwise).  Extra
    waits are hoisted onto EventSemaphore instructions inserted immediately
    before, on the same engine (same issue-gating semantics).
    """
    import json

    j = json.loads(bir_json_bytes)
    n_split = 0
    uid = [0]
    for f in j["functions"]:
        for b in f["blocks"]:
            out = []
            for ins in b["instructions"]:
                si = ins.get("sync_info")
                ow = (si or {}).get("on_wait") or []
                if len(ow) > 1 and ins.get("opcode") not in _NO_SPLIT_OPCODES:
                    for w in ow[:-1]:
                        uid[0] += 1
                        out.append(
                            {
                                "debug": ins.get("debug", 0),
                                "engine": ins["engine"],
                                "ins": [],
                                "name": f"{ins['name']}-wsplit{uid[0]}",
                                "opcode": "EventSemaphore",
                                "outs": [],
                                "sync_info": {"on_update": [], "on_wait": [w]},
                            }
                        )
                    si["on_wait"] = [ow[-1]]
                    n_split += 1
                out.append(ins)
            b["instructions"] = out
    return json.dumps(j).encode(), n_split


def build_nc(n_cores=N_CORES):
    """Build the per-core Bass program (SPMD: same program on every core)."""
    import concourse.bass as bass
    import concourse.tile as tile
    from concourse import mybir

    f32 = mybir.dt.float32
    bf16 = mybir.dt.bfloat16
    AOP = mybir.AluOpType
    ACT = mybir.ActivationFunctionType

    nc = bass.Bass(
        "TRN2", target_bir_lowering=False, debug=False, num_devices=n_cores
    )
    xs_d = nc.dram_tensor("xs", [P, F_TOT], bf16, kind="ExternalInput")
    out_d = nc.dram_tensor(
        "out", [N_STRIP, 3 * SLAB], f32, kind="ExternalOutput"
    )

    n_slab = F_TOT // SLAB  # 33
    # per (stream, strip) matmul totals for start/stop flags
    strip_total = [0] * N_STRIP
    for i in range(n_slab):
        strip_total[i % N_STRIP] += 1

    with tile.TileContext(nc) as tc:
        with (
            tc.tile_pool(name="const", bufs=1) as cpool,
            tc.tile_pool(name="big", bufs=1) as big,
            tc.tile_pool(name="fin", bufs=1) as fin,
            tc.tile_pool(name="acc", bufs=1, space="PSUM") as psum,
        ):
            ones = cpool.tile([P, 1], bf16)
            nc.vector.memset(ones[:], 1.0)
            eps_t = cpool.tile([P, 1], f32)
            nc.vector.memset(eps_t[:], EPS)

            # single resident tiles; DMA/compute operate on column regions
            xt = big.tile([P, F_TOT], bf16, name="xt")
            lx = big.tile([P, F_TOT], bf16, name="lx")
            sq = big.tile([P, F_TOT], bf16, name="sq")

            accs = [
                psum.tile([P, SLAB], f32, tag=f"acc{s}", name=f"acc{s}")
                for s in range(3)
            ]

            nmm = [[0] * N_STRIP for _ in range(3)]
            slab_idx = 0
            f0 = 0
            for ci, fm in enumerate(CHUNKS):
                cs = slice(f0, f0 + fm)
                nc.sync.dma_start(xt[:, cs], xs_d[:, cs])
                nc.scalar.activation(lx[:, cs], xt[:, cs], ACT.Ln, bias=eps_t[:])
                nc.vector.tensor_tensor(
                    sq[:, cs], lx[:, cs], lx[:, cs], AOP.mult
                )

                for j in range(fm // SLAB):
                    sl = slice(f0 + j * SLAB, f0 + (j + 1) * SLAB)
                    q = slab_idx % N_STRIP
                    for s, src in enumerate((xt, lx, sq)):
                        nc.tensor.matmul(
                            accs[s][32 * q : 32 * q + 1, :],
                            ones[:],
                            src[:, sl],
                            start=(nmm[s][q] == 0),
                            stop=(nmm[s][q] == strip_total[q] - 1),
                            tile_position=(0, 32 * q),
                        )
                        nmm[s][q] += 1
                    slab_idx += 1
                f0 += fm

            outsb = fin.tile([P, 3 * SLAB], f32)
            nc.vector.tensor_copy(outsb[:, 0:SLAB], accs[0][:, :])
            nc.scalar.activation(
                outsb[:, SLAB : 2 * SLAB], accs[1][:, :], ACT.Copy
            )
            nc.vector.tensor_copy(outsb[:, 2 * SLAB : 3 * SLAB], accs[2][:, :])
            nc.sync.dma_start(out_d[:], outsb[0:P:32, :])

    return nc


_PROG_CACHE = {}


def _get_prog():
    key = 0
    if key not in _PROG_CACHE:
        nc = build_nc()
        fixed, _n = _split_sync_waits(nc.to_json_bytes())
        nc.to_json_bytes = lambda: fixed
        _PROG_CACHE[key] = nc
    return _PROG_CACHE[key]


def _prepare(x, idx):
    """Sort/pad edges into per-core [128, F_TOT] |x| bf16 images.

    Returns (in_maps, host state dict for the finale).
    """
    import ml_dtypes

    x = np.asarray(x, dtype=np.float32).ravel()
    idx = np.asarray(idx).ravel().astype(np.int64)
    n = x.shape[0]

    neg = (x < 0).astype(np.int64)
    key = idx * 2 + neg
    order = np.argsort(key, kind="stable")
    xs = x[order]
    ks = key[order]
    gcnt = np.bincount(key, minlength=2 * NUM_SEG)
    gstart = np.zeros(2 * NUM_SEG, dtype=np.int64)
    np.cumsum(gcnt[:-1], out=gstart[1:])
    rank = np.arange(n, dtype=np.int64) - gstart[ks]
    ok = rank < C2

    flat = np.ones(2 * NUM_SEG * C2, dtype=np.float32)
    flat[ks[ok] * C2 + rank[ok]] = np.abs(xs[ok])

    # exact host-side corrections (float64)
    spill_x = np.zeros(NUM_SEG, dtype=np.float64)
    spill_l = np.zeros(NUM_SEG, dtype=np.float64)
    spill_q = np.zeros(NUM_SEG, dtype=np.float64)
    if not ok.all():
        sp = ~ok
        seg_sp = (ks[sp] >> 1).astype(np.int64)
        xv = xs[sp].astype(np.float64)
        lv = np.log(np.abs(xv) + EPS)
        np.add.at(spill_x, seg_sp, xv)
        np.add.at(spill_l, seg_sp, lv)
        np.add.at(spill_q, seg_sp, lv * lv)

    npad = (C2 - np.minimum(gcnt, C2)).astype(np.float64)  # [1024]
    counts = np.bincount(idx, minlength=NUM_SEG).astype(np.float64)

    flat16 = flat.astype(ml_dtypes.bfloat16)
    padded = flat16.reshape(NUM_SEG, 2, P, TPP)
    in_maps = []
    for c in range(N_CORES):
        a = padded[c * SEG_PER_CORE : (c + 1) * SEG_PER_CORE]  # [64,2,128,136]
        img = np.ascontiguousarray(
            a.transpose(2, 3, 0, 1).reshape(P, F_TOT)
        )
        in_maps.append({"xs": img})

    state = {
        "npad": npad.reshape(NUM_SEG, 2),
        "counts": counts,
        "spill": (spill_x, spill_l, spill_q),
    }
    return in_maps, state


def _fold_outputs(results):
    """Per-core [4, 1536] f32 -> [3, NUM_SEG, 2] (stream, seg, sign)."""
    sums = np.zeros((3, NUM_SEG, 2), dtype=np.float64)
    for c, res in enumerate(results):
        o = np.asarray(res["out"], dtype=np.float64)  # [4, 1536]
        for s in range(3):
            v = o[:, s * SLAB : (s + 1) * SLAB].sum(axis=0)  # [512]
            g = v.reshape(SLAB // N_GRP, N_GRP).sum(axis=0)  # [128] groups
            seg0 = c * SEG_PER_CORE
            sums[s, seg0 : seg0 + SEG_PER_CORE, :] = g.reshape(
                SEG_PER_CORE, 2
            )
    return sums


def _finale(sums, state, target_mean, target_std):
    npad = state["npad"]  # [512, 2]
    counts = state["counts"]
    spill_x, spill_l, spill_q = state["spill"]

    lpad = np.log1p(EPS)
    sx = (
        (sums[0, :, 0] - npad[:, 0])
        - (sums[0, :, 1] - npad[:, 1])
        + spill_x
    )
    sl = sums[1].sum(axis=1) - npad.sum(axis=1) * lpad + spill_l
    sq = sums[2].sum(axis=1) - npad.sum(axis=1) * lpad * lpad + spill_q

    cg = np.maximum(counts, 1.0)
    mean_w = sx / cg
    mean_log = sl / cg
    log_var = sq / cg - mean_log**2
    std_w = np.sqrt(log_var + EPS)
    tm = np.asarray(target_mean, dtype=np.float64)
    ts = np.asarray(target_std, dtype=np.float64)
    mean_loss = np.mean((mean_w - tm) ** 2)
    std_loss = np.mean((std_w - ts) ** 2)
    total = (1.0 - STD_WEIGHT) * mean_loss + STD_WEIGHT * std_loss
    return np.float32(total * STRENGTH)


def run_device(x, idx, trace=False):
    """Run the device program; returns (sums, state, res)."""
    _install_ntff_hook()
    from concourse.bass_utils import run_bass_kernel_spmd

    nc = _get_prog()
    in_maps, state = _prepare(x, idx)
    res = run_bass_kernel_spmd(
        nc, in_maps, list(range(N_CORES)), trace=trace
    )
    sums = _fold_outputs(res.results)
    return sums, state, res


def kernel(x, idx, target_mean, target_std):
    sums, state, _res = run_device(x, idx, trace=False)
    return _finale(sums, state, target_mean, target_std)


# revision 12
# speedup vs baseline: 26.9770x; 1.0006x over previous
"""Trainium2 Bass kernel for nn_MeanStdStiffRegularizer (segment reduce).

Strategy (8 NeuronCores, segment-sharded, sort-based):
  - The host shards BY SEGMENT: core c owns segments [64c, 64c+64).  Edges
    are permuted (stable sort by (segment, sign)) and padded so that each
    (segment, sign) group occupies a fixed [128, 136] block of the per-core
    [128, 17408] bf16 image; column f = t*128 + (2*seg_local + sign).
    Only |x| is shipped (sign is encoded in the column parity), so no idx
    tensor and no abs op on device.  Pads are 1.0 (ln(1+eps) ~ 0) and are
    subtracted exactly on the host.
  - Device per core: Ln(|x|+eps) on ScalarE (one 1x pass), L^2 on VectorE
    (bf16 tensor_tensor 2x), and all segment sums on the PE: matmul with a
    ones[128,1] stationary against 512-wide moving slabs, accumulated in
    PSUM.  Four PE column strips (tile_position) run concurrently; each
    PSUM column j accumulates group g = j%128.
  - No collective: each core returns 3x[128,512] f32 partials; the host
    folds strips/replicas and does the final 512-sized math in float64.
"""

import sys
import types

import numpy as np

N_EDGES = 16777216
NUM_SEG = 512
STRENGTH = 0.01
STD_WEIGHT = 0.5
EPS = 1e-6

N_CORES = 8
P = 128
SEG_PER_CORE = NUM_SEG // N_CORES  # 64
N_GRP = 2 * SEG_PER_CORE  # 128 (seg, sign) groups per core
TPP = 128  # elems per partition per (seg, sign) group
C2 = P * TPP  # 16384 capacity per (seg, sign) group (~0.3% spill to host)
F_TOT = N_GRP * TPP  # 16384 free elems per partition
SLAB = 512
N_STRIP = 4
# DMA/ACT chunk boundaries (slab-aligned; small first chunks for fast ramp,
# small last chunk for a short tail)
CHUNKS = (512, 1024, 2048, 4096, 4096, 4096, 512)
assert sum(CHUNKS) == F_TOT
N_EARLY_DMA = 2  # chunks issued from the Scalar engine (earlier preamble)


def _install_ntff_hook():
    """Register the axon NTFF profiling hook (missing antenv.axon_hooks)."""
    if "antenv.axon_hooks" in sys.modules:
        return
    mod = types.ModuleType("antenv.axon_hooks")
    _h = [None]
    mod.set_axon_ntff_profile_hook = lambda h: _h.__setitem__(0, h)
    mod.get_axon_ntff_profile_hook = lambda: _h[0]
    sys.modules["antenv.axon_hooks"] = mod
    try:
        from trn_agent_boot.trn_boot import _ntff_profile_via_ctypes

        mod.set_axon_ntff_profile_hook(
            _ntff_profile_via_ctypes("/opt/axon/libaxon_pjrt.so")
        )
    except Exception:
        pass


_NO_SPLIT_OPCODES = {
    "CollectiveCompute",
}


def _split_sync_waits(bir_json_bytes):
    """Rewrite BIR so no TPB instruction carries more than one sync wait.

    The walrus codegen in this container supports a single sync-wait slot
    per TPB instruction ("Too many sync wait commands" otherwise).  Extra
    waits are hoisted onto EventSemaphore instructions inserted immediately
    before, on the same engine (same issue-gating semantics).
    """
    import json

    j = json.loads(bir_json_bytes)
    n_split = 0
    uid = [0]
    for f in j["functions"]:
        for b in f["blocks"]:
            out = []
            for ins in b["instructions"]:
                si = ins.get("sync_info")
                ow = (si or {}).get("on_wait") or []
                if len(ow) > 1 and ins.get("opcode") not in _NO_SPLIT_OPCODES:
                    for w in ow[:-1]:
                        uid[0] += 1
                        out.append(
                            {
                                "debug": ins.get("debug", 0),
                                "engine": ins["engine"],
                                "ins": [],
                                "name": f"{ins['name']}-wsplit{uid[0]}",
                                "opcode": "EventSemaphore",
                                "outs": [],
                                "sync_info": {"on_update": [], "on_wait": [w]},
                            }
                        )
                    si["on_wait"] = [ow[-1]]
                    n_split += 1
                out.append(ins)
            b["instructions"] = out
    return json.dumps(j).encode(), n_split


def build_nc(n_cores=N_CORES):
    """Build the per-core Bass program (SPMD: same program on every core)."""
    import concourse.bass as bass
    import concourse.tile as tile
    from concourse import mybir

    f32 = mybir.dt.float32
    bf16 = mybir.dt.bfloat16
    AOP = mybir.AluOpType
    ACT = mybir.ActivationFunctionType

    nc = bass.Bass(
        "TRN2", target_bir_lowering=False, debug=False, num_devices=n_cores
    )
    xs_d = nc.dram_tensor("xs", [P, F_TOT], bf16, kind="ExternalInput")
    out_d = nc.dram_tensor(
        "out", [N_STRIP, 3 * SLAB], f32, kind="ExternalOutput"
    )

    with tile.TileContext(nc) as tc:
        with (
            tc.tile_pool(name="const", bufs=1) as cpool,
            tc.tile_pool(name="big", bufs=1) as big,
            tc.tile_pool(name="fin", bufs=1) as fin,
            tc.tile_pool(name="acc", bufs=1, space="PSUM") as psum,
        ):
            ones = cpool.tile([P, 1], bf16)
            nc.vector.memset(ones[:], 1.0)
            eps_t = cpool.tile([P, 1], f32)
            nc.vector.memset(eps_t[:], EPS)

            # single resident tiles; DMA/compute operate on column regions
            xt = big.tile([P, F_TOT], bf16, name="xt")
            lx = big.tile([P, F_TOT], bf16, name="lx")
            sq = big.tile([P, F_TOT], bf16, name="sq")

            accs = [
                psum.tile([P, SLAB], f32, tag=f"acc{s}", name=f"acc{s}")
                for s in range(3)
            ]

            # strip of the i-th emitted matmul = i % N_STRIP, so consecutive
            # PE instructions always hit different column strips (max
            # concurrency, incl. the tail).  Group (stream, strip) sizes:
            n_slab_total = F_TOT // SLAB
            group_total = [[0] * N_STRIP for _ in range(3)]
            for i in range(3 * n_slab_total):
                group_total[i % 3][i % N_STRIP] += 1

            # issue the first chunks' DMAs from the Scalar engine (its NEFF
            # preamble finishes earlier than Sync's), the rest from Sync
            f0 = 0
            for ci, fm in enumerate(CHUNKS):
                cs = slice(f0, f0 + fm)
                eng = nc.scalar if ci < N_EARLY_DMA else nc.sync
                eng.dma_start(xt[:, cs], xs_d[:, cs])
                f0 += fm

            nmm = [[0] * N_STRIP for _ in range(3)]
            mm_idx = 0
            f0 = 0
            for ci, fm in enumerate(CHUNKS):
                cs = slice(f0, f0 + fm)
                nc.scalar.activation(lx[:, cs], xt[:, cs], ACT.Ln, bias=eps_t[:])
                nc.vector.tensor_tensor(
                    sq[:, cs], lx[:, cs], lx[:, cs], AOP.mult
                )

                for j in range(fm // SLAB):
                    sl = slice(f0 + j * SLAB, f0 + (j + 1) * SLAB)
                    for s, src in enumerate((xt, lx, sq)):
                        q = mm_idx % N_STRIP
                        nc.tensor.matmul(
                            accs[s][32 * q : 32 * q + 1, :],
                            ones[:],
                            src[:, sl],
                            start=(nmm[s][q] == 0),
                            stop=(nmm[s][q] == group_total[s][q] - 1),
                            tile_position=(0, 32 * q),
                        )
                        nmm[s][q] += 1
                        mm_idx += 1
                f0 += fm

            outsb = fin.tile([P, 3 * SLAB], f32)
            nc.vector.tensor_copy(outsb[:, 0:SLAB], accs[0][:, :])
            nc.scalar.activation(
                outsb[:, SLAB : 2 * SLAB], accs[1][:, :], ACT.Copy
            )
            nc.vector.tensor_copy(outsb[:, 2 * SLAB : 3 * SLAB], accs[2][:, :])
            nc.sync.dma_start(out_d[:], outsb[0:P:32, :])

    return nc


_PROG_CACHE = {}


def _get_prog():
    key = 0
    if key not in _PROG_CACHE:
        nc = build_nc()
        fixed, _n = _split_sync_waits(nc.to_json_bytes())
        nc.to_json_bytes = lambda: fixed
        _PROG_CACHE[key] = nc
    return _PROG_CACHE[key]


def _prepare(x, idx):
    """Sort/pad edges into per-core [128, F_TOT] |x| bf16 images.

    Returns (in_maps, host state dict for the finale).
    """
    import ml_dtypes

    x = np.asarray(x, dtype=np.float32).ravel()
    idx = np.asarray(idx).ravel().astype(np.int64)
    n = x.shape[0]

    neg = (x < 0).astype(np.int64)
    key = idx * 2 + neg
    order = np.argsort(key, kind="stable")
    xs = x[order]
    ks = key[order]
    gcnt = np.bincount(key, minlength=2 * NUM_SEG)
    gstart = np.zeros(2 * NUM_SEG, dtype=np.int64)
    np.cumsum(gcnt[:-1], out=gstart[1:])
    rank = np.arange(n, dtype=np.int64) - gstart[ks]
    ok = rank < C2

    flat = np.ones(2 * NUM_SEG * C2, dtype=np.float32)
    flat[ks[ok] * C2 + rank[ok]] = np.abs(xs[ok])

    # exact host-side corrections (float64)
    spill_x = np.zeros(NUM_SEG, dtype=np.float64)
    spill_l = np.zeros(NUM_SEG, dtype=np.float64)
    spill_q = np.zeros(NUM_SEG, dtype=np.float64)
    if not ok.all():
        sp = ~ok
        seg_sp = (ks[sp] >> 1).astype(np.int64)
        xv = xs[sp].astype(np.float64)
        lv = np.log(np.abs(xv) + EPS)
        np.add.at(spill_x, seg_sp, xv)
        np.add.at(spill_l, seg_sp, lv)
        np.add.at(spill_q, seg_sp, lv * lv)

    npad = (C2 - np.minimum(gcnt, C2)).astype(np.float64)  # [1024]
    counts = np.bincount(idx, minlength=NUM_SEG).astype(np.float64)

    flat16 = flat.astype(ml_dtypes.bfloat16)
    padded = flat16.reshape(NUM_SEG, 2, P, TPP)
    in_maps = []
    for c in range(N_CORES):
        a = padded[c * SEG_PER_CORE : (c + 1) * SEG_PER_CORE]  # [64,2,128,136]
        img = np.ascontiguousarray(
            a.transpose(2, 3, 0, 1).reshape(P, F_TOT)
        )
        in_maps.append({"xs": img})

    state = {
        "npad": npad.reshape(NUM_SEG, 2),
        "counts": counts,
        "spill": (spill_x, spill_l, spill_q),
    }
    return in_maps, state


def _fold_outputs(results):
    """Per-core [4, 1536] f32 -> [3, NUM_SEG, 2] (stream, seg, sign)."""
    sums = np.zeros((3, NUM_SEG, 2), dtype=np.float64)
    for c, res in enumerate(results):
        o = np.asarray(res["out"], dtype=np.float64)  # [4, 1536]
        for s in range(3):
            v = o[:, s * SLAB : (s + 1) * SLAB].sum(axis=0)  # [512]
            g = v.reshape(SLAB // N_GRP, N_GRP).sum(axis=0)  # [128] groups
            seg0 = c * SEG_PER_CORE
            sums[s, seg0 : seg0 + SEG_PER_CORE, :] = g.reshape(
                SEG_PER_CORE, 2
            )
    return sums


def _finale(sums, state, target_mean, target_std):
    npad = state["npad"]  # [512, 2]
    counts = state["counts"]
    spill_x, spill_l, spill_q = state["spill"]

    lpad = np.log1p(EPS)
    sx = (
        (sums[0, :, 0] - npad[:, 0])
        - (sums[0, :, 1] - npad[:, 1])
        + spill_x
    )
    sl = sums[1].sum(axis=1) - npad.sum(axis=1) * lpad + spill_l
    sq = sums[2].sum(axis=1) - npad.sum(axis=1) * lpad * lpad + spill_q

    cg = np.maximum(counts, 1.0)
    mean_w = sx / cg
    mean_log = sl / cg
    log_var = sq / cg - mean_log**2
    std_w = np.sqrt(log_var + EPS)
    tm = np.asarray(target_mean, dtype=np.float64)
    ts = np.asarray(target_std, dtype=np.float64)
    mean_loss = np.mean((mean_w - tm) ** 2)
    std_loss = np.mean((std_w - ts) ** 2)
    total = (1.0 - STD_WEIGHT) * mean_loss + STD_WEIGHT * std_loss
    return np.float32(total * STRENGTH)


def run_device(x, idx, trace=False):
    """Run the device program; returns (sums, state, res)."""
    _install_ntff_hook()
    from concourse.bass_utils import run_bass_kernel_spmd

    nc = _get_prog()
    in_maps, state = _prepare(x, idx)
    res = run_bass_kernel_spmd(
        nc, in_maps, list(range(N_CORES)), trace=trace
    )
    sums = _fold_outputs(res.results)
    return sums, state, res


def kernel(x, idx, target_mean, target_std):
    sums, state, _res = run_device(x, idx, trace=False)
    return _finale(sums, state, target_mean, target_std)
